# revision 1
# baseline (speedup 1.0000x reference)
"""Trainium2 Bass kernel for nn_Down_channelV2 (Mamba cross-modal block).

Sharding: 8 cores = batch (2) x d_inner-shard (4 x 144). Per core:
  - LayerNorm + W_in matmuls + depthwise conv + x_dbl partial (bf16 matmuls)
  - AllReduce x_dbl partials within each batch's 4-core group
  - selective scan via native DVE tensor_tensor_scan in (d,n)-on-partitions
    layout (18 tiles of 128 states x L); dA args built by PE matmuls, exp on ACT
  - n-contraction via PE indicator matmul, W_out partial, raw-reshape to the
    pixel layout through DRAM, fc1 partial (linear -> folded before the reduce)
  - ReduceScatter of fc1 partials, then LayerNorm + GELU on the owned quarter.
Self-contained: hardcodes all shapes from the problem spec.
"""
import sys

sys.path.insert(0, "/opt/trn_rl_repo")

import numpy as np

import concourse.bass as bass
import concourse.bacc as bacc
import concourse.mybir as mybir
from concourse import tile

F32 = np.float32
DT = mybir.dt
OP = mybir.AluOpType
ACTF = mybir.ActivationFunctionType

Bt, Cm, Hh, Ww = 2, 96, 64, 64
LFULL = Hh * Ww
C, D, N, R, KW, OUT = 288, 576, 16, 18, 3, 96
NCORES = 8
DS = D // 4          # 144 channels per core
NT = DS * N // 128   # 18 scan tiles per core
XD = R + 2 * N       # 50
EPS = 1e-5


# ---------------------------------------------------------------------------
# host-side prep: pure layout work (slice / transpose / concat / 0-1 masks)
# ---------------------------------------------------------------------------
def host_shards(inputs, L=LFULL):
    x1, x2, x3 = inputs['x1'], inputs['x2'], inputs['x3']
    W_in, W_x, W_dt, W_out = inputs['W_in'], inputs['W_x'], inputs['W_dt'], inputs['W_out']
    m16 = np.zeros((DS, 128), F32)
    for d in range(DS):
        m16[d, (d % 8) * 16:(d % 8) * 16 + 16] = 1.0
    g16 = np.zeros((128, 8), F32)
    for p in range(128):
        g16[p, p // 16] = 1.0
    ones96 = np.ones((96, 1), F32)
    LQ = L // 4
    shards = []
    for c in range(NCORES):
        b, s = c // 4, c % 4
        ds = slice(s * DS, (s + 1) * DS)
        xrow = np.concatenate(
            [x1[b].reshape(-1), x2[b].reshape(-1), x3[b].reshape(-1)]
        ).reshape(LFULL, C)[:L]
        xpix = xrow.reshape(-1).reshape(C, L) if L == LFULL else \
            np.ascontiguousarray(xrow).reshape(-1).reshape(C, L)
        sh = dict(
            xT=xrow.T,                                            # [C, L] f32
            xqpix=xpix[:, s * LQ:(s + 1) * LQ],                   # [C, LQ]
            win_xi=W_in[:, ds],                                   # [C, DS]
            win_z=W_in[:, D + s * DS: D + (s + 1) * DS],
            wx=W_x[ds, :],                                        # [DS, 50]
            wdt=W_dt[:, ds],                                      # [R, DS]
            wout=W_out[ds, :],                                    # [DS, C]
            fc1w=inputs['fc1_w'],                                 # [C, OUT]
            convw=inputs['conv_w'][ds, :],                        # [DS, 3]
            convb=inputs['conv_b'][ds].reshape(DS, 1),
            bdt=inputs['b_dt'][ds].reshape(DS, 1),
            dssm=inputs['D_ssm'][ds].reshape(DS, 1),
            alog=inputs['A_log'][ds, :],                          # [DS, N]
            ln0g=inputs['ln0_g'].reshape(3, 96).T,                # [96, 3]
            ln0b=inputs['ln0_b'].reshape(3, 96).T,
            fc1b=inputs['fc1_b'].reshape(OUT, 1),
            ln1g=inputs['ln1_g'].reshape(OUT, 1),
            ln1b=inputs['ln1_b'].reshape(OUT, 1),
            m16=m16, g16=g16, ones96=ones96,
        )
        shards.append({k: np.ascontiguousarray(v, dtype=F32) for k, v in sh.items()})
    return shards


def input_shapes(L):
    LQ = L // 4
    return dict(
        xT=(C, L), xqpix=(C, LQ), win_xi=(C, DS), win_z=(C, DS), wx=(DS, XD),
        wdt=(R, DS), wout=(DS, C), fc1w=(C, OUT), convw=(DS, 3), convb=(DS, 1),
        bdt=(DS, 1), dssm=(DS, 1), alog=(DS, N), ln0g=(96, 3), ln0b=(96, 3),
        fc1b=(OUT, 1), ln1g=(OUT, 1), ln1b=(OUT, 1), m16=(DS, 128), g16=(128, 8),
        ones96=(96, 1),
    )


class Split:
    """A DS=144-row tensor as two sbuf tiles: [128, F] + [16, F]."""

    def __init__(self, pool, F, dtype, tag):
        self.t = [pool.tile([128, F], dtype, tag=tag + "_a", name=tag + "_a"),
                  pool.tile([16, F], dtype, tag=tag + "_b", name=tag + "_b")]

    def parts(self):
        return [(self.t[0], 0, 128), (self.t[1], 128, 16)]

    def rows(self, r0, n):
        if r0 + n <= 128:
            return self.t[0][r0:r0 + n]
        assert r0 >= 128
        return self.t[1][r0 - 128:r0 - 128 + n]


def build(tc, io, L):
    nc = tc.nc
    SCH = min(512, L)          # psum free chunk
    NSC = L // SCH
    ACH = min(1024, L)         # exp / scan chunk
    NAC = L // ACH
    LQ = L // 4
    f32, bf16 = DT.float32, DT.bfloat16

    def ln_stats(pool, src_tiles, nchan, Lx, pref):
        """LayerNorm stats over partition groups via PE ones-matmul.

        src_tiles: list of [96, Lx] bf16 tiles covering nchan rows.
        Returns (rstd_bc, mrs_bc) [96, Lx] bf16 broadcast tiles."""
        nk = len(src_tiles)
        nsc = Lx // min(512, Lx)
        sch = min(512, Lx)
        LR = Lx // 128
        stf_d = nc.dram_tensor(pref + "_stf", [2 * Lx], DT.float32)
        stb_d = nc.dram_tensor(pref + "_stb", [2 * Lx], DT.bfloat16)
        with tc.tile_pool(name=pref + "ps", bufs=4, space="PSUM") as pp, \
             tc.tile_pool(name=pref + "sq", bufs=2) as sqp:
            for ch in range(nsc):
                sl = slice(ch * sch, (ch + 1) * sch)
                ps1 = pp.tile([1, sch], f32, tag="ps1", name="ps1")
                ps2 = pp.tile([1, sch], f32, tag="ps2", name="ps2")
                for k in range(nk):
                    xsq = sqp.tile([96, sch], bf16, tag="xsq", name="xsq")
                    nc.vector.tensor_tensor(xsq[:], src_tiles[k][:, sl],
                                            src_tiles[k][:, sl], OP.mult)
                    nc.tensor.matmul(ps1[:], ones96b[:], src_tiles[k][:, sl],
                                     start=(k == 0), stop=(k == nk - 1))
                    nc.tensor.matmul(ps2[:], ones96b[:], xsq[:],
                                     start=(k == 0), stop=(k == nk - 1))
                c1 = sqp.tile([1, sch], f32, tag="c1", name="c1")
                c2 = sqp.tile([1, sch], f32, tag="c2", name="c2")
                nc.vector.tensor_copy(c1[:], ps1[:])
                nc.vector.tensor_copy(c2[:], ps2[:])
                nc.sync.dma_start(
                    stf_d[ch * sch:(ch + 1) * sch].rearrange("(o f) -> o f", o=1),
                    c1[:])
                nc.sync.dma_start(
                    stf_d[Lx + ch * sch:Lx + (ch + 1) * sch].rearrange(
                        "(o f) -> o f", o=1), c2[:])
        st = pool.tile([128, 4 * LR], f32, tag=pref + "st")
        mu, ms, var, mrs = (st[:, i * LR:(i + 1) * LR] for i in range(4))
        nc.sync.dma_start(mu, stf_d[0:Lx].rearrange("(p f) -> p f", p=128))
        nc.sync.dma_start(ms, stf_d[Lx:2 * Lx].rearrange("(p f) -> p f", p=128))
        nc.scalar.mul(mu, mu, 1.0 / nchan)
        nc.scalar.mul(ms, ms, 1.0 / nchan)
        musq = pool.tile([128, LR], f32, tag=pref + "musq")
        nc.vector.tensor_tensor(musq[:], mu, mu, OP.mult)
        nc.vector.tensor_tensor(var, ms, musq[:], OP.subtract)
        nc.scalar.activation(var, var, ACTF.Ln, bias=epsc[:])
        nc.scalar.activation(var, var, ACTF.Exp, scale=-0.5)      # var <- rstd
        nc.vector.scalar_tensor_tensor(mrs, mu, -1.0, var, OP.mult, OP.mult)
        stb = pool.tile([128, 2 * LR], bf16, tag=pref + "stb")
        nc.vector.tensor_copy(stb[:, :LR], var)
        nc.vector.tensor_copy(stb[:, LR:], mrs)
        nc.sync.dma_start(stb_d[0:Lx].rearrange("(p f) -> p f", p=128), stb[:, :LR])
        nc.sync.dma_start(stb_d[Lx:2 * Lx].rearrange("(p f) -> p f", p=128), stb[:, LR:])
        r_bc = pool.tile([96, Lx], bf16, tag=pref + "rbc")
        m_bc = pool.tile([96, Lx], bf16, tag=pref + "mbc")
        nc.sync.dma_start(r_bc[0:1], stb_d[0:Lx].rearrange("(o f) -> o f", o=1))
        nc.sync.dma_start(m_bc[0:1], stb_d[Lx:2 * Lx].rearrange("(o f) -> o f", o=1))
        k = 1
        while k < 96:
            n2 = min(k, 96 - k)
            nc.sync.dma_start(r_bc[k:k + n2], r_bc[0:n2])
            nc.sync.dma_start(m_bc[k:k + n2], m_bc[0:n2])
            k += n2
        return r_bc, m_bc

    # ======== persistent pools ========
    cpool = tc.alloc_tile_pool(name="consts", bufs=1)
    wpool = tc.alloc_tile_pool(name="work", bufs=1)

    def loadc(name, pool=None, bf=False):
        src = io[name]
        p, f = src.shape
        t = (pool or cpool).tile([p, f], f32, tag=name)
        nc.sync.dma_start(t[:], src[:])
        if not bf:
            return t
        tb = (pool or cpool).tile([p, f], bf16, tag=name + "_bf")
        nc.vector.tensor_copy(tb[:], t[:])
        return tb

    def loadS(name, F, bf=False):
        sp = Split(cpool, F, f32, name)
        for t_, r0, nr in sp.parts():
            nc.sync.dma_start(t_[:], io[name][r0:r0 + nr])
        if not bf:
            return sp
        sb = Split(cpool, F, bf16, name + "_bf")
        for (t_, _, _), (tb, _, _) in zip(sp.parts(), sb.parts()):
            nc.vector.tensor_copy(tb[:], t_[:])
        return sb

    # ---- constants & weights ----
    ln0g = loadc('ln0g'); ln0b = loadc('ln0b')
    fc1b = loadc('fc1b'); ln1g = loadc('ln1g'); ln1b = loadc('ln1b')
    ones96b = loadc('ones96', bf=True)
    g16b = loadc('g16', bf=True)
    wdtb = loadc('wdt', bf=True)                    # [18, DS]
    def load_rows_bf(name, k, F, tmp_pool):
        tf = tmp_pool.tile([96, F], f32, tag="ldtmp", name="ldtmp")
        nc.sync.dma_start(tf[:], io[name][96 * k:96 * (k + 1)])
        tb = cpool.tile([96, F], bf16, tag=f"{name}{k}", name=f"{name}{k}")
        nc.vector.tensor_copy(tb[:], tf[:])
        return tb

    fc1wb, winxib, winzb = [], [], []
    with tc.tile_pool(name="ldtmp", bufs=2) as ltp:
        for k in range(3):
            fc1wb.append(load_rows_bf('fc1w', k, OUT, ltp))
            winxib.append(load_rows_bf('win_xi', k, DS, ltp))
            winzb.append(load_rows_bf('win_z', k, DS, ltp))
    wxb = loadS('wx', XD, bf=True)
    woutb = loadS('wout', C, bf=True)
    convw = loadS('convw', 3)
    convb = loadS('convb', 1)
    bdt = loadS('bdt', 1)
    dssm = loadS('dssm', 1)
    alog = loadS('alog', N)
    m16 = loadS('m16', 128)
    epsc = cpool.tile([128, 1], f32, tag="eps", name="eps")
    nc.vector.memset(epsc[:], EPS)

    # repna[d, p] = A[d, p%16] * (p//16 == d%8), A = -exp(alog)
    repna = Split(cpool, 128, bf16, "repna")
    for (al, r0, nr), (mk, _, _), (rp, _, _) in zip(
            alog.parts(), m16.parts(), repna.parts()):
        A_ = cpool.tile([nr, N], f32, tag=f"A{nr}", name=f"A{nr}")
        nc.scalar.activation(A_[:], al[:], ACTF.Exp)
        nc.vector.tensor_scalar_mul(A_[:], A_[:], -1.0)
        ar = cpool.tile([nr, 128], f32, tag=f"arep{nr}", name=f"arep{nr}")
        nc.sync.dma_start(ar[:], A_[:].unsqueeze(1).broadcast_to((nr, 8, N)))
        nc.vector.tensor_tensor(ar[:], ar[:], mk[:], OP.mult)
        nc.vector.tensor_copy(rp[:], ar[:])
    # block-diagonal lhsT per scan tile: repnaF[t][k, p] nonzero only for
    # k in [8t, 8t+8) -- keeps matmul operand base_partition at 0.
    repnaF = []
    for t in range(NT):
        r0 = 8 * t
        kk = 128 if t < 16 else 16
        rf = cpool.tile([kk, 128], bf16, tag=f"repnaF{t}", name=f"repnaF{t}")
        nc.vector.memset(rf[:], 0.0)
        rloc = r0 if t < 16 else r0 - 128
        nc.sync.dma_start(rf[rloc:rloc + 8], repna.rows(r0, 8))
        repnaF.append(rf)

    # ---- internal DRAM ----
    res_d = nc.dram_tensor("res_d", [L * C], DT.bfloat16)
    ar_in = nc.dram_tensor("ar_in", [XD * L], DT.bfloat16)
    ar_out = nc.dram_tensor("ar_out", [XD * L], DT.bfloat16)
    rs_in = nc.dram_tensor("rs_in", [4 * OUT * LQ], DT.bfloat16)
    rs_out = nc.dram_tensor("rs_out", [OUT * LQ], DT.bfloat16)

    def pixv(ap):   # [C(cc), L(lp)] pixel-layout view of a flat [L*C] buffer
        return ap.rearrange("(cc lp) -> cc lp", lp=L)

    def mamv(ap):   # [c, l] mamba-layout view of the same buffer
        return ap.rearrange("(l c) -> c l", c=C)

    # persistent activations
    zsil = Split(wpool, L, bf16, "zsil")
    xcb = Split(wpool, L, bf16, "xcb")
    dtbf = Split(wpool, L, bf16, "dtbf")
    bbc = wpool.tile([128, L], bf16, tag="bbc", name="bbc")
    ccb = wpool.tile([128, L], bf16, tag="ccb", name="ccb")
    u = Split(wpool, L, bf16, "u")
    yT = Split(wpool, L, bf16, "yT")

    # ================= Phase 1-2: LN0 + xi/z =================
    pconv = tc.alloc_tile_pool(name="pconv", bufs=1)
    xiT = Split(pconv, L + 2, bf16, "xiT")
    with tc.tile_pool(name="ph1", bufs=1) as p1:
        xbf = []
        with tc.tile_pool(name="xload", bufs=2) as xlp:
            for k in range(3):
                xf = xlp.tile([96, L], f32, tag="xf", name="xf")
                nc.sync.dma_start(xf[:], io['xT'][96 * k:96 * (k + 1)])
                xb = p1.tile([96, L], bf16, tag=f"xbf{k}", name=f"xbf{k}")
                nc.vector.tensor_copy(xb[:], xf[:])
                xbf.append(xb)
        rstd_bc, mrs_bc = ln_stats(p1, xbf, C, L, "ln0")
        xn = []
        for k in range(3):
            t = xbf[k]  # normalize in place; raw x not needed afterwards
            nc.vector.tensor_tensor(t[:], t[:], rstd_bc[:], OP.mult)
            nc.vector.tensor_tensor(t[:], t[:], mrs_bc[:], OP.add)
            nc.vector.tensor_scalar(t[:], t[:], ln0g[:, k:k + 1],
                                    ln0b[:, k:k + 1], OP.mult, OP.add)
            xn.append(t)

        for t_, _, _ in xiT.parts():
            nc.vector.memset(t_[:, 0:2], 0.0)
        with tc.tile_pool(name="mm_ps", bufs=2, space="PSUM") as pp, \
             tc.tile_pool(name="mm_sb", bufs=2) as pp_sb:
            for (xit, r0, nr), (zt, _, _) in zip(xiT.parts(), zsil.parts()):
                for ch in range(NSC):
                    sl = slice(ch * SCH, (ch + 1) * SCH)
                    psx = pp.tile([nr, SCH], f32, tag=f"psx{nr}", name=f"psx{nr}")
                    psz = pp.tile([nr, SCH], f32, tag=f"psz{nr}", name=f"psz{nr}")
                    for k in range(3):
                        nc.tensor.matmul(psx[:], winxib[k][:, r0:r0 + nr],
                                         xn[k][:, sl], start=(k == 0), stop=(k == 2))
                        nc.tensor.matmul(psz[:], winzb[k][:, r0:r0 + nr],
                                         xn[k][:, sl], start=(k == 0), stop=(k == 2))
                    nc.vector.tensor_copy(
                        xit[:, 2 + ch * SCH:2 + (ch + 1) * SCH], psx[:])
                    zraw = pp_sb.tile([nr, SCH], bf16, tag=f"zraw{nr}", name=f"zraw{nr}")
                    zsg = pp_sb.tile([nr, SCH], bf16, tag=f"zsg{nr}", name=f"zsg{nr}")
                    nc.vector.tensor_copy(zraw[:], psz[:])
                    nc.scalar.activation(zsg[:], psz[:], ACTF.Sigmoid)
                    nc.vector.tensor_tensor(zt[:, sl], zraw[:], zsg[:], OP.mult)

    # ================= Phase 3: conv + silu -> xc =================
    with tc.tile_pool(name="conv_t", bufs=2) as cvp:
        for (xit, r0, nr), (xct, _, _) in zip(xiT.parts(), xcb.parts()):
            t1 = cvp.tile([nr, L], bf16, tag=f"cv{nr}", name=f"cv{nr}")
            nc.vector.tensor_scalar_mul(t1[:], xit[:, 0:L], convw.rows(r0, nr)[:, 0:1])
            nc.vector.scalar_tensor_tensor(
                t1[:], xit[:, 1:L + 1], convw.rows(r0, nr)[:, 1:2], t1[:],
                OP.mult, OP.add)
            nc.vector.scalar_tensor_tensor(
                t1[:], xit[:, 2:L + 2], convw.rows(r0, nr)[:, 2:3], t1[:],
                OP.mult, OP.add)
            nc.vector.tensor_scalar_add(t1[:], t1[:], convb.rows(r0, nr))
            csg = cvp.tile([nr, L], bf16, tag=f"csg{nr}", name=f"csg{nr}")
            nc.scalar.activation(csg[:], t1[:], ACTF.Sigmoid)
            nc.vector.tensor_tensor(xct[:], t1[:], csg[:], OP.mult)

    pconv.release()

    # ================= Phase 4: x_dbl partial + AllReduce =================
    pxda = tc.alloc_tile_pool(name="pxda", bufs=1)
    xda = pxda.tile([XD, L], bf16, tag="xda", name="xda")
    xdp = pxda.tile([XD, L], bf16, tag="xdp", name="xdp")
    with tc.tile_pool(name="xd_ps", bufs=2, space="PSUM") as pp, \
         tc.tile_pool(name="xd_sb", bufs=2) as sb:
        for ch in range(NSC):
            sl = slice(ch * SCH, (ch + 1) * SCH)
            ps = pp.tile([XD, SCH], f32, tag="psxd", name="psxd")
            for i, (xct, r0, nr) in enumerate(xcb.parts()):
                nc.tensor.matmul(ps[:], wxb.parts()[i][0][:], xct[:, sl],
                                 start=(i == 0), stop=(i == 1))
            nc.vector.tensor_copy(xdp[:, sl], ps[:])
    LH = L // 2
    arv_in = ar_in[:].rearrange("(h p f) -> h p f", h=2, p=XD)
    arv_out = ar_out[:].rearrange("(h p f) -> h p f", h=2, p=XD)
    with tc.tile_pool(name="dt_ps", bufs=2, space="PSUM") as pp:
        for hf in range(2):
            hs = slice(hf * LH, (hf + 1) * LH)
            nc.sync.dma_start(arv_in[hf], xdp[:, hs])
            nc.gpsimd.collective_compute(
                "AllReduce", OP.add,
                replica_groups=[[0, 1, 2, 3], [4, 5, 6, 7]],
                ins=[arv_in[hf]], outs=[arv_out[hf]])
            nc.sync.dma_start(xda[:, hs], arv_out[hf])
            # -------- dt / B / C / u for this half --------
            SC2 = min(SCH, LH)
            for dtt, r0, nr in dtbf.parts():
                for ch in range(LH // SC2):
                    sl = slice(hf * LH + ch * SC2, hf * LH + (ch + 1) * SC2)
                    ps = pp.tile([nr, SC2], f32, tag=f"psdt{nr}", name=f"psdt{nr}")
                    nc.tensor.matmul(ps[:], wdtb[:, r0:r0 + nr], xda[0:R, sl],
                                     start=True, stop=True)
                    # softplus(x) = ln(1+exp(x)); x ~= -4.6, never overflows
                    nc.scalar.activation(ps[:], ps[:], ACTF.Exp,
                                         bias=bdt.rows(r0, nr))
                    nc.scalar.activation(dtt[:, sl], ps[:], ACTF.Ln, bias=1.0)
            for g in range(8):
                nc.sync.dma_start(bbc[16 * g:16 * (g + 1), hs], xda[R:R + N, hs])
                nc.sync.dma_start(ccb[16 * g:16 * (g + 1), hs], xda[R + N:R + 2 * N, hs])
            for (ut, r0, nr), (dtt, _, _), (xct, _, _) in zip(
                    u.parts(), dtbf.parts(), xcb.parts()):
                nc.vector.tensor_tensor(ut[:, hs], dtt[:, hs], xct[:, hs], OP.mult)
    pxda.release()

    # ================= Phase 8: main scan loop =================
    with tc.tile_pool(name="sc_ps", bufs=2, space="PSUM") as app, \
         tc.tile_pool(name="g_ps", bufs=4, space="PSUM") as gpp, \
         tc.tile_pool(name="sc_w2", bufs=3) as skp, \
         tc.tile_pool(name="sc_w1", bufs=1) as skp1:
        SCC = min(2048, L)   # scan / dA chunk
        NSCC = L // SCC

        def scan_tail(r0, h):
            # n-contraction for a finished scan tile; emitted one iteration
            # late so it never sits between two scans in the DVE queue.
            # hc overwrites h in place (h is dead after this).
            Lh = L // 2
            nc.gpsimd.tensor_tensor(h[:, Lh:], h[:, Lh:], ccb[:, Lh:], OP.mult)
            nc.vector.tensor_tensor(h[:, :Lh], h[:, :Lh], ccb[:, :Lh], OP.mult)
            ystg = skp1.tile([8, L], bf16, tag="ystg", name="ystg")
            for ch in range(NSC):
                sl = slice(ch * SCH, (ch + 1) * SCH)
                gps = gpp.tile([8, SCH], f32, tag="gps", name="gps")
                nc.tensor.matmul(gps[:], g16b[:], h[:, sl],
                                 start=True, stop=True)
                nc.scalar.copy(ystg[:, sl], gps[:])
            nc.sync.dma_start(yT.rows(r0, 8)[:], ystg[:])

        pending = None
        for t in range(NT):
            r0 = 8 * t
            urep = skp.tile([128, L], bf16, tag="urep", name="urep")
            dtt = dtbf.t[0] if t < 16 else dtbf.t[1]
            dBu = urep  # in place: urep is dead once dBu is formed
            Lh2 = L // 2
            for hf2 in range(2):
                hs2 = slice(hf2 * Lh2, (hf2 + 1) * Lh2)
                nc.sync.dma_start(
                    urep[:, hs2],
                    u.rows(r0, 8)[:, hs2].unsqueeze(1).broadcast_to((8, N, Lh2)))
                nc.gpsimd.tensor_tensor(dBu[:, hs2], urep[:, hs2], bbc[:, hs2],
                                        OP.mult)
            h = skp.tile([128, L], bf16, tag="h", name="h")
            for qc in range(NSCC):
                dA = skp.tile([128, SCC], f32, tag="dA", name="dA")
                for q in range(SCC // ACH):
                    ps = app.tile([128, ACH], f32, tag="psA", name="psA")
                    for j in range(ACH // SCH):
                        sl = slice(qc * SCC + q * ACH + j * SCH,
                                   qc * SCC + q * ACH + (j + 1) * SCH)
                        nc.tensor.matmul(ps[:, j * SCH:(j + 1) * SCH],
                                         repnaF[t][:], dtt[:, sl],
                                         start=True, stop=True)
                    nc.scalar.activation(dA[:, q * ACH:(q + 1) * ACH], ps[:],
                                         ACTF.Exp)
                sl = slice(qc * SCC, (qc + 1) * SCC)
                init = 0.0 if qc == 0 else h[:, qc * SCC - 1:qc * SCC]
                nc.vector.tensor_tensor_scan(
                    h[:, sl], dA[:], dBu[:, sl], init, OP.mult, OP.add)
            if pending is not None:
                scan_tail(*pending)
            pending = (r0, h)
        scan_tail(*pending)

    # ================= Phase 9-11: y2, res->pix, fc1 partial, RS ==========
    with tc.tile_pool(name="post", bufs=1) as po:
        with tc.tile_pool(name="y2_t", bufs=2) as yp:
            y2 = Split(po, L, bf16, "y2")
            for (y2t, r0, nr), (yt, _, _), (xct, _, _), (zt, _, _) in zip(
                    y2.parts(), yT.parts(), xcb.parts(), zsil.parts()):
                x2 = yp.tile([nr, L], bf16, tag=f"x2_{nr}", name=f"x2_{nr}")
                nc.vector.tensor_scalar_mul(x2[:], xct[:], dssm.rows(r0, nr))
                nc.vector.tensor_tensor(y2t[:], yt[:], x2[:], OP.add)
                nc.vector.tensor_tensor(y2t[:], y2t[:], zt[:], OP.mult)

        # res partial in [l-part, c-free] layout (y2 stationary, wout moving)
        # so the DRAM write of the raw (l,c)-major flat buffer is contiguous.
        resv = res_d[:].rearrange("(l c) -> l c", c=C)
        with tc.tile_pool(name="rs_ps", bufs=4, space="PSUM") as pp, \
             tc.tile_pool(name="rs_sb", bufs=4) as sb:
            for lt in range(L // 128):
                ls = slice(lt * 128, (lt + 1) * 128)
                ps = pp.tile([128, C], f32, tag="psr", name="psr")
                for i, (y2t, r0, nr) in enumerate(y2.parts()):
                    nc.tensor.matmul(
                        ps[:], y2t[:, ls], woutb.parts()[i][0][:],
                        start=(i == 0), stop=(i == 1))
                ot = sb.tile([128, C], bf16, tag="resb", name="resb")
                nc.vector.tensor_copy(ot[:], ps[:])
                nc.sync.dma_start(resv[ls], ot[:])

        pixp = []
        for j in range(3):
            t = po.tile([96, L], bf16, tag=f"pixp{j}", name=f"pixp{j}")
            nc.sync.dma_start(t[:], pixv(res_d[:])[96 * j:96 * (j + 1)])
            pixp.append(t)

        qp = po.tile([OUT, L], bf16, tag="qp", name="qp")
        with tc.tile_pool(name="q_ps", bufs=2, space="PSUM") as pp:
            for ch in range(NSC):
                sl = slice(ch * SCH, (ch + 1) * SCH)
                ps = pp.tile([OUT, SCH], f32, tag="psq", name="psq")
                for k in range(3):
                    nc.tensor.matmul(ps[:], fc1wb[k][:], pixp[k][:, sl],
                                     start=(k == 0), stop=(k == 2))
                nc.vector.tensor_copy(qp[:, sl], ps[:])
        rsv = rs_in[:].rearrange("(r p f) -> r p f", r=4, p=OUT)
        for r in range(4):
            nc.sync.dma_start(rsv[r], qp[:, r * LQ:(r + 1) * LQ])
    nc.gpsimd.collective_compute(
        "ReduceScatter", OP.add,
        replica_groups=[[0, 1, 2, 3], [4, 5, 6, 7]],
        ins=[rs_in[:]], outs=[rs_out[:]])

    # ================= Phase 12: tail on owned quarter =================
    with tc.tile_pool(name="tail", bufs=1) as tp:
        qsum = tp.tile([OUT, LQ], bf16, tag="qsum", name="qsum")
        nc.sync.dma_start(qsum[:], rs_out[:].rearrange("(p f) -> p f", p=OUT))
        qsumf = tp.tile([OUT, LQ], f32, tag="qsumf", name="qsumf")
        nc.vector.tensor_copy(qsumf[:], qsum[:])
        xq = []
        with tc.tile_pool(name="xqld", bufs=2) as xlp:
            for j in range(3):
                xf = xlp.tile([96, LQ], f32, tag="xqf", name="xqf")
                nc.sync.dma_start(xf[:], io['xqpix'][96 * j:96 * (j + 1)])
                xb = tp.tile([96, LQ], bf16, tag=f"xq{j}", name=f"xq{j}")
                nc.vector.tensor_copy(xb[:], xf[:])
                xq.append(xb)
        pre = tp.tile([OUT, LQ], f32, tag="pre", name="pre")
        nsc = LQ // min(512, LQ)
        sch = min(512, LQ)
        with tc.tile_pool(name="tx_ps", bufs=2, space="PSUM") as pp:
            for ch in range(nsc):
                sl = slice(ch * sch, (ch + 1) * sch)
                ps = pp.tile([OUT, sch], f32, tag="pst", name="pst")
                for k in range(3):
                    nc.tensor.matmul(ps[:], fc1wb[k][:], xq[k][:, sl],
                                     start=(k == 0), stop=(k == 2))
                nc.vector.tensor_copy(pre[:, sl], ps[:])
        nc.vector.tensor_tensor(pre[:], pre[:], qsumf[:], OP.add)
        nc.vector.tensor_scalar_add(pre[:], pre[:], fc1b[:])
        preb = tp.tile([OUT, LQ], bf16, tag="preb", name="preb")
        nc.vector.tensor_copy(preb[:], pre[:])
        r1bc, m1bc = ln_stats(tp, [preb], OUT, LQ, "ln1")
        r1f = tp.tile([OUT, LQ], f32, tag="r1f", name="r1f")
        m1f = tp.tile([OUT, LQ], f32, tag="m1f", name="m1f")
        nc.vector.tensor_copy(r1f[:], r1bc[:])
        nc.vector.tensor_copy(m1f[:], m1bc[:])
        nc.vector.tensor_tensor(pre[:], pre[:], r1f[:], OP.mult)
        nc.vector.tensor_tensor(pre[:], pre[:], m1f[:], OP.add)
        nc.vector.tensor_scalar(pre[:], pre[:], ln1g[:], ln1b[:], OP.mult, OP.add)
        # gelu(x) ~= 0.5 x (1 + tanh(0.7978845608 (x + 0.044715 x^3)))
        outt = tp.tile([OUT, LQ], f32, tag="outt", name="outt")
        gsq = tp.tile([OUT, LQ], f32, tag="gsq", name="gsq")
        nc.scalar.activation(gsq[:], pre[:], ACTF.Square)
        nc.vector.scalar_tensor_tensor(gsq[:], gsq[:], 0.044715, pre[:],
                                       OP.mult, OP.mult)
        nc.vector.tensor_tensor(gsq[:], gsq[:], pre[:], OP.add)
        nc.scalar.activation(gsq[:], gsq[:], ACTF.Tanh, scale=0.7978845608)
        nc.vector.tensor_scalar(gsq[:], gsq[:], 0.5, 0.5, OP.mult, OP.add)
        nc.vector.tensor_tensor(outt[:], pre[:], gsq[:], OP.mult)
        nc.sync.dma_start(io['out'][:], outt[:])

    wpool.release()
    cpool.release()


# ---------------------------------------------------------------------------
# program construction + host entry
# ---------------------------------------------------------------------------
def make_program(L=LFULL):
    nc = bacc.Bacc("TRN2", target_bir_lowering=False, debug=False, num_devices=NCORES)
    io = {}
    for k, shp in input_shapes(L).items():
        io[k] = nc.dram_tensor(k, list(shp), DT.float32, kind="ExternalInput").ap()
    io['out'] = nc.dram_tensor("out", [OUT, L // 4], DT.float32,
                               kind="ExternalOutput").ap()
    with tile.TileContext(nc) as tc:
        build(tc, io, L)
    nc.compile()
    return nc


_PROG = {}


LAST_EXEC_NS = None
LAST_RESULTS = None


def kernel(_trace=False, **inputs):
    global LAST_EXEC_NS, LAST_RESULTS
    inputs = {k: np.asarray(v) for k, v in inputs.items()}
    L = LFULL
    if L not in _PROG:
        _PROG[L] = make_program(L)
    nc = _PROG[L]
    shards = host_shards(inputs, L)
    from concourse.bass_utils import run_bass_kernel_spmd
    res = run_bass_kernel_spmd(nc, shards, list(range(NCORES)), trace=_trace)
    LAST_RESULTS = res
    if res.exec_time_ns is not None:
        LAST_EXEC_NS = res.exec_time_ns
    out = np.zeros((Bt, OUT, LFULL), F32)
    LQ = L // 4
    for c in range(NCORES):
        b, r = c // 4, c % 4
        out[b][:, r * LQ:(r + 1) * LQ] = res.results[c]['out']
    return out.reshape(Bt, OUT, Hh, Ww)


if __name__ == '__main__':
    print("kernel module - use kernel(**inputs)")

